# revision 45
# baseline (speedup 1.0000x reference)
"""Trainium2 Bass kernel for BiLevelRoutingAttention (nn_BiLevelRoutingAttention_66907000537867).

Sharding: one attention head per NeuronCore (8 heads / 8 cores). v2 design:

  phase 1: qkv projection in bf16 (host-cast xT halves the input DMA),
           evacuated in ONE ACT op per tile with the bias fused (Identity
           activation + per-partition bias AP) into a persistent [96, NPIX]
           bf16 stage A (q|k|v). v is also scattered into VT band 0 (padded
           image layout) and computed again in pixel-major layout directly on
           the PE (xt chunk stationary x w_v) into v_aug with a ones column
           (the DMA xbar transpose corrupts data for some SBUF address
           configurations, so it is not used). k (and a q replica) are also
           staged at partition bases 0/32 for QK^T row-group packing.
  bands:   VT [128, 114x116] holds 4 bands: v shifted by 0/1/2 image rows
           (DVE 4x flat copies per window-row) + out_cm (normalized attn).
           Column shifts of the 3x3 lepe conv come from the rhs AP offset, so
           the 9 taps + attn tap collapse into 3 accumulating K>=96 matmuls.
  phase 2: per region (49): QK^T (bf16, 2 halves in PE row-groups 0/1 via
           tile_position so they run concurrently on HW) -> exp (ScalarE,
           scale fused) -> attn@V with a ones column giving denominators in
           psum row 32 (software-pipelined one region behind the exps so the
           in-order PE queue never stalls); one DVE evac [33,256] to bf16;
           denominators spread across partitions by tiny DMAs; per-batch
           reciprocal; broadcast back via stride-0 DMA; DVE mul writes
           normalized attn into VT band 3 (image layout).
  phase 3: interleaved with phase 2 (no barriers): per 4 image rows and
           output half, a 3-matmul accumulation group (G_dx0 K=96, G_dx1
           K=128 incl. attn tap, G_dx2 K=96) -> evac -> store; groups are
           released as soon as their window-row is normalized.

Host: window-ordering of pixels, region routing (the mean commutes exactly
with the linear qkv layer), per-head weight slicing, lepe taps folded into
w_o column blocks, final sum of per-core partials + constant bias row.
"""

import numpy as np
import ml_dtypes

import concourse.bass as bass
import concourse.bacc as bacc
import concourse.mybir as mybir
import concourse.tile as tile
from concourse.tile import add_dep_helper
from concourse.bass_utils import run_bass_kernel_spmd

F32 = mybir.dt.float32
F32R = mybir.dt.float32r
BF16 = mybir.dt.bfloat16
AF = mybir.ActivationFunctionType

DIM, QK, HEADS, NWIN, TOPK = 256, 256, 8, 7, 4
H = W = 112
P2 = NWIN * NWIN          # 49 regions
W2 = 256                  # pixels per region (16x16)
NPIX = H * W              # 12544
HD = 32                   # per-head dim
SCALE = QK ** (-0.5)      # 1/16
NT = 25                   # pixel tiles: 24x512 + 1x256
VW = 116                  # padded image row stride (4B-aligned shifts)
VR = 114                  # padded image rows
VTW = VR * VW             # 13224
N3 = 448                  # phase-3 pixel tile (4 image rows)

_cache = {}


def _tile_w(t):
    return 512 if t < 24 else 256


def _build(top_idx, has_vbias=True, debug=False):
    nc = bacc.Bacc()
    xT_d = nc.declare_dram_parameter("xT", [DIM, NPIX], BF16, isOutput=False)
    wqkv_d = nc.declare_dram_parameter("wqkv", [DIM, 96], BF16, isOutput=False)
    bqkv_d = nc.declare_dram_parameter("bqkv", [96, 1], F32, isOutput=False)
    wt_d = nc.declare_dram_parameter("wt", [128, 6 * 128], BF16, isOutput=False)
    out_d = nc.declare_dram_parameter("out", [DIM, NPIX], F32, isOutput=True)

    with tile.TileContext(nc) as tc, tc.tile_pool(name="persist", bufs=1) as persist:
        # ---- persistent SBUF ----
        w_sb = persist.tile([128, 192], BF16)         # qkv weights, 2 cin chunks
        bqkv_sb = persist.tile([96, 1], F32)
        A = persist.tile([96, NPIX], BF16)            # q 0-31 | k 32-63 | v 64-95
        kx = persist.tile([32, NPIX], BF16)           # k copy at partition base 0
        q2 = persist.tile([64, NPIX], BF16)           # q replica at partition base 32
        v_aug = persist.tile([128, 98, 34], BF16)     # pixel-major v + ones col 32
        bias_v = persist.tile([128, 32], F32)         # b_qkv[v] broadcast (host)
        VT = persist.tile([128, VTW], BF16)           # dy0|dy1|dy2 v bands + out_cm
        ou = persist.tile([33, NPIX], BF16)           # unnorm attn (ch-major) + den row
        den = persist.tile([49, 256], BF16)           # denominators, one region/partition
        rec = persist.tile([49, 256], BF16)           # reciprocals
        rec1 = persist.tile([1, NPIX], BF16)          # reciprocals gathered to part 0
        wt_sb = persist.tile([128, 6 * 128], BF16)    # phase-3 stationaries

        bv_d = nc.declare_dram_parameter("bv", [128, 32], F32, isOutput=False)
        nc.sync.dma_start(out=w_sb[:, 0:96], in_=wqkv_d[0:128, :])
        nc.sync.dma_start(out=w_sb[:, 96:192], in_=wqkv_d[128:256, :])
        nc.sync.dma_start(out=bqkv_sb, in_=bqkv_d[:, :])
        if has_vbias:
            nc.sync.dma_start(out=bias_v, in_=bv_d[:, :])
        nc.vector.memset(v_aug[:, :, 32:33], 1.0)
        # den rows are reciprocal'd [0:49] per batch before every row is
        # written (DVE needs 32-aligned start partitions); 1.0 is harmless
        nc.vector.memset(den, 1.0)

        VTv = VT.rearrange("p (r c) -> p r c", c=VW)
        # zero only band 0's padding border (interiors overwritten by the
        # window copies; bands 1-2 are copied from band 0; band 3 reads
        # touch written positions only)
        nc.vector.memset(VTv[0:32, 0, :], 0.0)
        nc.vector.memset(VTv[0:32, 113, :], 0.0)
        nc.vector.memset(VTv[0:32, 1:113, 0:2], 0.0)
        nc.vector.memset(VTv[0:32, 1:113, 114:116], 0.0)

        # ---- phase 1: qkv projection (f32r) ----
        with (
            tc.tile_pool(name="xt", bufs=3) as xtp,
            tc.tile_pool(name="qkv_ps", bufs=2, space="PSUM") as qkvps,
            tc.tile_pool(name="vp_ps", bufs=2, space="PSUM") as vpsp,
            tc.tile_pool(name="dum_ps", bufs=2, space="PSUM") as dumps,
        ):
            # pre-observe each DMA semaphore with a tiny dummy matmul ordered
            # before the real one so the real matmul needs <=1 sync wait.
            def observe(aps):
                dum = dumps.tile([1, 1], F32, tag="dum")
                last = None
                for ap in aps:
                    d = nc.tensor.matmul(dum, ap[0:1, 0:1], ap[0:1, 0:1],
                                         start=True, stop=True)
                    if last is not None:
                        add_dep_helper(d.ins, last.ins, sync=False)
                    last = d
                return last

            bands_wr = 0
            for t in range(NT):
                w = _tile_w(t)
                n0 = 512 * t
                xt0 = xtp.tile([128, w], BF16, tag="xt0")
                xt1 = xtp.tile([128, w], BF16, tag="xt1")
                nc.sync.dma_start(out=xt0, in_=xT_d[0:128, n0:n0 + w])
                nc.sync.dma_start(out=xt1, in_=xT_d[128:256, n0:n0 + w])
                obs = observe([w_sb, xt0, xt1] if t == 0 else [xt0, xt1])
                ps = qkvps.tile([96, w], F32, tag="qkv")
                m1 = nc.tensor.matmul(ps, w_sb[:, 0:96], xt0,
                                      start=True, stop=False)
                add_dep_helper(m1.ins, obs.ins, sync=False)
                nc.tensor.matmul(ps, w_sb[:, 96:192], xt1,
                                 start=False, stop=True)
                # single-op evacuation with bias fused, f32 -> bf16 on ACT
                nc.scalar.activation(A[:, n0:n0 + w], ps, AF.Identity,
                                     bias=bqkv_sb, scale=1.0)
                # v -> VT band 0 (padded image; pixel (R,C) at (R+1, C+2))
                Av = A.rearrange("p (a b c) -> p a b c", b=16, c=16)
                for wi in range(w // 256):
                    win = 2 * t + wi
                    wr, wc = divmod(win, NWIN)
                    nc.vector.tensor_copy(
                        VTv[0:32, 16 * wr + 1:16 * wr + 17,
                            16 * wc + 2:16 * wc + 18],
                        Av[64:96, win, :, :])
                # v in pixel-major layout directly on PE: xt chunk as the
                # stationary x^T [cin, pix], w_v as the moving operand.
                # (dma_start_transpose silently corrupts data for some SBUF
                # address configurations on HW — avoid it.)
                for ci in range(w // 128):
                    c = 4 * t + ci
                    vps = vpsp.tile([128, 32], F32, tag="vps")
                    nc.tensor.matmul(vps, xt0[:, 128 * ci:128 * (ci + 1)],
                                     w_sb[:, 64:96], start=True, stop=False)
                    nc.tensor.matmul(vps, xt1[:, 128 * ci:128 * (ci + 1)],
                                     w_sb[:, 160:192], start=False, stop=True)
                    if has_vbias:
                        nc.vector.tensor_add(v_aug[:, c, 0:32], vps, bias_v)
                    elif c % 2 == 0:
                        nc.scalar.copy(v_aug[:, c, 0:32], vps)
                    else:
                        nc.vector.tensor_copy(v_aug[:, c, 0:32], vps)
                # k copy to partition base 0 (matmul lhsT/rhs bases must
                # match); incremental to stay off the phase-2 critical path
                nc.vector.tensor_copy(kx[:, n0:n0 + w], A[32:64, n0:n0 + w])
                # q replica at base 32 so QK^T half 1 runs in PE row-group 1
                # (tile_position packing; concurrent with half 0 on HW)
                nc.sync.dma_start(out=q2[32:64, n0:n0 + w],
                                  in_=A[0:32, n0:n0 + w])
                # row-shifted v bands (column shifts come from the rhs AP);
                # band rows of window-row wr also need the first rows of
                # window-row wr+1, so copy wr once wr+1 is fully written
                windows_done = 2 * t + w // 256
                while bands_wr < NWIN and (
                        windows_done >= NWIN * (bands_wr + 2)
                        or windows_done == P2):
                    r0 = 16 * bands_wr * VW
                    r1 = (16 * bands_wr + 16) * VW
                    nc.vector.tensor_copy(VT[32:64, r0:r1],
                                          VT[0:32, r0 + VW:r1 + VW])
                    nc.vector.tensor_copy(VT[64:96, r0:r1],
                                          VT[0:32, r0 + 2 * VW:r1 + 2 * VW])
                    bands_wr += 1

        # phase-3 stationaries: not needed until the first p3 group, keep the
        # load off the startup DMA critical path
        nc.sync.dma_start(out=wt_sb, in_=wt_d[:, :])

        # ---- phase 2 + 3 interleaved ----
        ouv = ou.rearrange("p (a b c) -> p a b c", b=16, c=16)
        with (
            tc.tile_pool(name="attnT_ps", bufs=2, space="PSUM") as atps,
            tc.tile_pool(name="outT_ps", bufs=1, space="PSUM") as otps,
            tc.tile_pool(name="acc_ps", bufs=2, space="PSUM") as accps,
            tc.tile_pool(name="expT", bufs=4) as expp,
            tc.tile_pool(name="bcs", bufs=4) as bcsp,
            tc.tile_pool(name="ev", bufs=3) as evp,
        ):
            def p3_group(n, hh):
                acc = accps.tile([128, N3], F32, tag="acc")
                for dx in range(3):
                    kk = 128 if dx == 1 else 96
                    rhs = VTv[0:kk, 4 * n:4 * n + 4, dx + 1:dx + 113]
                    nc.tensor.matmul(acc, wt_sb[0:kk, 128 * (2 * dx + hh):
                                                128 * (2 * dx + hh + 1)],
                                     rhs, start=(dx == 0), stop=(dx == 2))
                ev = evp.tile([128, N3], F32, tag="ev")
                nc.vector.tensor_copy(ev, acc)
                nc.sync.dma_start(
                    out=out_d[128 * hh:128 * (hh + 1), N3 * n:N3 * (n + 1)],
                    in_=ev)

            batch_lo = 0
            p3_next = 0
            pending = []
            prev = None

            def emit_pending(k):
                for _ in range(min(k, len(pending))):
                    n, hh = pending.pop(0)
                    p3_group(n, hh)

            def finish_region(r, chunks, exs):
                # attn@V for a region whose exps were issued one iteration
                # earlier (software pipelining keeps the in-order PE queue
                # from stalling on the exp -> attn@V dependency)
                nonlocal batch_lo, p3_next
                ot = otps.tile([33, W2], F32, tag="ot")
                for j in range(8):
                    nc.tensor.matmul(ot, v_aug[:, chunks[j], 0:33],
                                     exs[j // 4][:, 256 * (j % 4):256 * (j % 4 + 1)],
                                     start=(j == 0), stop=(j == 7))
                # evac attn rows + den row to bf16 in one DVE op
                nc.vector.tensor_copy(ou[:, W2 * r:W2 * (r + 1)], ot)
                # spread this region's denominators to partition r
                nc.sync.dma_start(out=den[r:r + 1, :],
                                  in_=ou[32:33, W2 * r:W2 * (r + 1)])

                if r in (16, 32, 41, 45, 48):
                    lo, hi = batch_lo, r + 1
                    batch_lo = hi
                    with nc.allow_low_precision(reason="bf16 softmax denominators"):
                        nc.vector.reciprocal(rec, den)
                    nc.sync.dma_start(out=rec1[0:1, W2 * lo:W2 * hi],
                                      in_=rec[lo:hi, :])
                    for rr in range(lo, hi):
                        bcs = bcsp.tile([32, W2], BF16, tag="bcs")
                        nc.sync.dma_start(
                            out=bcs,
                            in_=rec1[0:1, W2 * rr:W2 * (rr + 1)]
                            .rearrange("p (a n) -> p a n", a=1)
                            .broadcast_to([1, 32, W2]))
                        wr_, wc_ = divmod(rr, NWIN)
                        nc.vector.tensor_mul(
                            VTv[96:128, 16 * wr_:16 * wr_ + 16,
                                16 * wc_ + 2:16 * wc_ + 18],
                            ouv[0:32, rr, :, :],
                            bcs.rearrange("p (a b) -> p a b", b=16))
                    # queue phase-3 groups whose window-row is fully normalized
                    while p3_next < 28 and 7 * (p3_next // 4) + 7 <= hi:
                        for hh in range(2):
                            pending.append((p3_next, hh))
                        p3_next += 1
                emit_pending(3)

            prev = None
            for r in range(P2):
                chunks = [2 * g + jj for g in top_idx[r] for jj in (0, 1)]
                q_ap = A[0:32, W2 * r:W2 * (r + 1)]
                exs = []
                for half in range(2):
                    # half 0 contracts in PE row-group 0 (k/q at base 0),
                    # half 1 in row-group 1 (k/q at base 32) — on HW the two
                    # K=32 groups run concurrently via tile_position
                    ksrc = kx if half == 0 else A[32:64, :]
                    qsrc = q_ap if half == 0 else q2[32:64,
                                                     W2 * r:W2 * (r + 1)]
                    at = atps.tile([128, 1024], F32, tag="at")
                    for j4 in range(4):
                        c = chunks[4 * half + j4]
                        nc.tensor.matmul(at[:, 256 * j4:256 * (j4 + 1)],
                                         ksrc[:, 128 * c:128 * (c + 1)],
                                         qsrc, start=True, stop=True)
                    ex = expp.tile([128, 1024], BF16, tag="ex")
                    nc.scalar.activation(ex, at, AF.Exp, scale=SCALE)
                    exs.append(ex)
                if prev is not None:
                    finish_region(*prev)
                prev = (r, chunks, exs)
            finish_region(*prev)
            emit_pending(len(pending))

        if debug:
            tc.strict_bb_all_engine_barrier()
            dbg = {
                "dbg_A": A, "dbg_kx": kx, "dbg_vaug": v_aug, "dbg_VT": VT,
                "dbg_ou": ou, "dbg_den": den, "dbg_rec1": rec1,
            }
            for name, t in dbg.items():
                sh = [t.shape[0], int(np.prod(t.shape[1:]))]
                d = nc.declare_dram_parameter(name, sh, t.dtype, isOutput=True)
                nc.sync.dma_start(out=d[:, :], in_=t.rearrange(
                    "p ... -> p (...)") if len(t.shape) > 2 else t[:, :])
    nc.compile()
    return nc


def _host_prep(x, w_qkv, b_qkv):
    xw = x.reshape(NWIN, 16, NWIN, 16, DIM).transpose(0, 2, 1, 3, 4)
    xw = np.ascontiguousarray(xw.reshape(NPIX, DIM))
    xT = np.ascontiguousarray(xw.T)
    xmean = xw.reshape(P2, W2, DIM).mean(1)
    q_win = xmean @ w_qkv[:, :QK] + b_qkv[:QK]
    k_win = xmean @ w_qkv[:, QK:2 * QK] + b_qkv[QK:2 * QK]
    logit = (q_win * SCALE) @ k_win.T
    top_idx = np.argsort(-logit, axis=-1, kind="stable")[:, :TOPK]
    return xT, top_idx


def _in_maps(x, w_qkv, b_qkv, w_o, lepe_w):
    xT, top_idx = _host_prep(x[0], w_qkv, b_qkv)
    xT_bf = np.ascontiguousarray(xT.astype(ml_dtypes.bfloat16))
    lw = lepe_w[:, :, 0, :]  # [3,3,256]
    maps = []
    for h in range(HEADS):
        sl = slice(h * HD, (h + 1) * HD)
        wqkv_h = np.concatenate(
            [w_qkv[:, :QK][:, sl], w_qkv[:, QK:2 * QK][:, sl],
             w_qkv[:, 2 * QK:][:, sl]], axis=1)
        bqkv_h = np.concatenate(
            [b_qkv[:QK][sl], b_qkv[QK:2 * QK][sl], b_qkv[2 * QK:][sl]])
        w_o_h = w_o[sl, :]  # [32, 256]
        wt_h = np.zeros((128, 6 * 128), np.float32)
        for dx in range(3):
            for hh in range(2):
                blk = slice(128 * (2 * dx + hh), 128 * (2 * dx + hh + 1))
                for dy in range(3):
                    wt_h[32 * dy:32 * dy + 32, blk] = \
                        lw[dy, dx, sl][:, None] * w_o_h[:, 128 * hh:128 * hh + 128]
                if dx == 1:
                    wt_h[96:128, blk] = w_o_h[:, 128 * hh:128 * hh + 128]
        maps.append({
            "xT": xT_bf,
            "wqkv": np.ascontiguousarray(wqkv_h.astype(ml_dtypes.bfloat16)),
            "bqkv": np.ascontiguousarray(bqkv_h[:, None]),
            "bv": np.ascontiguousarray(
                np.broadcast_to(bqkv_h[64:96][None, :], (128, 32)).astype(np.float32)),
            "wt": np.ascontiguousarray(wt_h.astype(ml_dtypes.bfloat16)),
        })
    return maps, top_idx


def kernel(x, w_qkv, b_qkv, w_o, b_o, lepe_w, lepe_b):
    x = np.asarray(x, np.float32)
    w_qkv = np.asarray(w_qkv, np.float32)
    b_qkv = np.asarray(b_qkv, np.float32)
    w_o = np.asarray(w_o, np.float32)
    b_o = np.asarray(b_o, np.float32)
    lepe_w = np.asarray(lepe_w, np.float32)
    lepe_b = np.asarray(lepe_b, np.float32)

    maps, top_idx = _in_maps(x, w_qkv, b_qkv, w_o, lepe_w)
    has_vbias = bool(np.any(b_qkv[2 * QK:]))
    key = (top_idx.tobytes(), has_vbias)
    if key not in _cache:
        _cache[key] = _build(top_idx, has_vbias)
    nc = _cache[key]

    res = run_bass_kernel_spmd(nc, maps, list(range(HEADS))).results
    total = np.zeros((DIM, NPIX), np.float32)
    for h in range(HEADS):
        total += np.asarray(res[h]["out"], np.float32)
    b_all = lepe_b @ w_o + b_o
    out = total.T + b_all
    return out.reshape(1, H, W, DIM).astype(np.float32)
